# revision 36
# baseline (speedup 1.0000x reference)
"""Multi-head self-attention (B=2, S=2048, D=1024, H=16, causal) on 8 NeuronCores.

Sharding: core c = 4*b + g handles batch b and heads 4g..4g+3 (batch x
head-group parallel). Per core (all matmul operands bf16, fp32 PSUM accum):
  - q/k projections in transposed layout  qT/kT [dh, s]  (dh on partitions)
  - v projection in natural layout [s, dh] with a fused ones-column per head
    (gives the softmax denominator for free during the AV matmul)
  - causal attention in scoresT [j, i] orientation, single-chunk [128, 512]
    score tiles: PE scores -> ACT exp (scale=1/8, no max subtraction) ->
    DVE causal mask multiply on diagonal chunks -> PE AV accumulate.
    Emission is software-pipelined at CHUNK granularity: the AV stream of
    step g-1 and projection/o-proj blocks are zipped between the score
    matmuls of step g, so the in-order PE never waits for ACT's exp stream
    and the HAM clock gate stays released.
  - normalization of attnT by the per-query denominator via DVE reciprocal
    -> GPSIMD partition_broadcast -> DVE multiply into mergedT (bf16)
  - partial o-projection out_c = merged_c @ Wo[:, cols_c].T, bf16 staging
Host sums the 4 bf16 partial outputs per batch in f32 (the only cross-core
reduction).

bf16 matmuls run at 1 cycle/row on the TRN2 PE (fp32 modes are 2x slower)
and enable the compiler's fast-weight-load path for 128-col stationaries.
"""

import numpy as np
import ml_dtypes

import concourse.bass as bass
from concourse import bacc
import concourse.mybir as mybir
import concourse.tile as tile
from concourse import bass_utils

F32 = mybir.dt.float32
BF16 = mybir.dt.bfloat16
EXP = mybir.ActivationFunctionType.Exp
BF = ml_dtypes.bfloat16

B, S, D = 2, 2048, 1024
H, DH = 16, 64
NCORES = 8
HPG = 4                  # heads per group (per core)
M = HPG * DH             # 256 per-core head dims
DC = D // 128            # 8 contraction chunks for projections
IC = 512                 # i (query) chunk for attention
JC = 128                 # j (key) chunk for attention
SCALE = 1.0 / np.sqrt(DH)


def _build_nc():
    nc = bacc.Bacc("TRN2", target_bir_lowering=False, debug=False)

    xT_d = nc.dram_tensor("xT", [D, S], BF16, kind="ExternalInput").ap()
    wqkv_d = nc.dram_tensor("wqkvT", [D, 3 * M], BF16, kind="ExternalInput").ap()
    woT_d = nc.dram_tensor("woT", [M, D], BF16, kind="ExternalInput").ap()
    mask_d = nc.dram_tensor("mask", [JC, 4 * IC], BF16, kind="ExternalInput").ap()
    onesb_d = nc.dram_tensor("ones_b", [JC, HPG], BF16, kind="ExternalInput").ap()
    out_d = nc.dram_tensor("out", [S, D], BF16, kind="ExternalOutput").ap()

    with tile.TileContext(nc) as tc:
        _body(tc, xT_d, wqkv_d, woT_d, mask_d, onesb_d, out_d)
    nc.compile()
    return nc


def _body(tc, xT_d, wqkv_d, woT_d, mask_d, onesb_d, out_d):
    nc = tc.nc
    from contextlib import ExitStack
    ctx = ExitStack()
    with ctx:
        p_x = ctx.enter_context(tc.tile_pool(name="x", bufs=DC))
        p_w = ctx.enter_context(tc.tile_pool(name="w", bufs=DC))
        p_wo = ctx.enter_context(tc.tile_pool(name="wo", bufs=2))
        p_qk = ctx.enter_context(tc.tile_pool(name="qk", bufs=16))
        p_v = ctx.enter_context(tc.tile_pool(name="v", bufs=S // JC))
        p_mg = ctx.enter_context(tc.tile_pool(name="mg", bufs=2))
        p_probs = ctx.enter_context(tc.tile_pool(name="probs", bufs=36))
        p_small = ctx.enter_context(tc.tile_pool(name="small", bufs=4))
        p_bc = ctx.enter_context(tc.tile_pool(name="bc", bufs=4))
        p_mask = ctx.enter_context(tc.tile_pool(name="mask", bufs=1))
        p_ostg = ctx.enter_context(tc.tile_pool(name="ostg", bufs=2))
        p_ones = ctx.enter_context(tc.tile_pool(name="ones", bufs=1))

        ps_at = ctx.enter_context(tc.tile_pool(name="psa", bufs=3, space="PSUM"))
        ps_sc = ctx.enter_context(tc.tile_pool(name="pss", bufs=3, space="PSUM"))
        ps_pr = ctx.enter_context(tc.tile_pool(name="psp", bufs=2, space="PSUM"))

        # ---- HAM pre-warm: a burst of discarded matmuls while the first
        # x/w tiles stream in keeps the PE activity monitor busy so the
        # clock gate is released when the real projections start.
        NWARM = 32
        wrm = p_ones.tile([128, 512], BF16, tag="warm")
        nc.vector.memset(wrm[:], 1.0)
        wrm_ps = ps_at.tile([128, 512], F32, tag="attn", name="warmps")
        for r in range(NWARM):
            nc.tensor.matmul(wrm_ps[:], wrm[:, 0:128], wrm[:],
                             start=(r == 0), stop=(r == NWARM - 1))
        nc.scalar.copy(wrm[:, 0:1], wrm_ps[:, 0:1])  # keep alive vs DCE
        # preload the Exp activation table while the PE warms up (the first
        # real exp would otherwise pay the ~1.3us table load mid-pipeline)
        nc.scalar.activation(wrm[:, 1:2], wrm_ps[:, 1:2], EXP, scale=SCALE)

        # ---- input loads, in the order the projection matmuls consume them
        w_t, x_t = [], []
        for dc in range(DC):
            wt = p_w.tile([128, 3 * M], BF16, tag="w")
            nc.sync.dma_start(wt[:], wqkv_d[dc * 128:(dc + 1) * 128, :])
            w_t.append(wt)
            xt = p_x.tile([128, S], BF16, tag="x")
            nc.sync.dma_start(xt[:], xT_d[dc * 128:(dc + 1) * 128, :])
            x_t.append(xt)
        wo_t = []
        for kc in range(2):
            t = p_wo.tile([128, D], BF16, tag="wo")
            nc.sync.dma_start(t[:], woT_d[kc * 128:(kc + 1) * 128, :])
            wo_t.append(t)
        mask_t = p_mask.tile([JC, 4 * IC], BF16, tag="mask")
        nc.sync.dma_start(mask_t[:], mask_d[:])
        onesb_t = p_ones.tile([JC, HPG], BF16, tag="onesb")
        nc.sync.dma_start(onesb_t[:], onesb_d[:])

        # ---- projection building blocks ----
        q_t = {(mc, s4): p_qk.tile([128, 512], BF16, tag="qk",
                                    name=f"qT{mc}_{s4}")
               for mc in range(2) for s4 in range(4)}
        k_t = {(mc, s4): p_qk.tile([128, 512], BF16, tag="qk",
                                    name=f"kT{mc}_{s4}")
               for mc in range(2) for s4 in range(4)}
        mg_t = [p_mg.tile([128, S], BF16, tag="mgT", name=f"mg{i}")
                for i in range(M // 128)]

        def qk_block(tg, mc, s4, split):
            # qT/kT [m, s] = sum_d WT[d, m] xT[d, s]; m-chunk mc, s-chunk s4.
            woff = 0 if tg == "q" else M
            dst = (q_t if tg == "q" else k_t)[(mc, s4)]
            sl = slice(s4 * 512, (s4 + 1) * 512)
            wsl = slice(woff + mc * 128, woff + (mc + 1) * 128)
            ps = ps_pr.tile([128, 512], F32, tag="proj")
            for dc in range(DC):
                nc.tensor.matmul(ps[:], w_t[dc][:, wsl], x_t[dc][:, sl],
                                 start=(dc == 0), stop=(dc == DC - 1))
            nc.vector.tensor_copy(dst[:], ps[:])

        def qk_half(tg, mc, s4, half):
            # Half-contraction (dc 0-3 or 4-7) evicted immediately: lets the
            # PE run on the first-arrived x/w tiles during the DMA ramp.
            woff = 0 if tg == "q" else M
            dst = (q_t if tg == "q" else k_t)[(mc, s4)]
            sl = slice(s4 * 512, (s4 + 1) * 512)
            wsl = slice(woff + mc * 128, woff + (mc + 1) * 128)
            dcs = range(DC // 2) if half == 0 else range(DC // 2, DC)
            ps = ps_pr.tile([128, 512], F32, tag="proj")
            for u, dc in enumerate(dcs):
                nc.tensor.matmul(ps[:], w_t[dc][:, wsl], x_t[dc][:, sl],
                                 start=(u == 0), stop=(u == DC // 2 - 1))
            if half == 0:
                nc.vector.tensor_copy(dst[:], ps[:])
            else:
                nc.vector.tensor_add(dst[:], dst[:], ps[:])

        v_t = {}

        def v_block(sc):
            # v[s, m] tile for j-chunk sc: per head h cols h*65..h*65+63 = v,
            # col h*65+64 = 1.0 (softmax denominator column)
            vt = p_v.tile([JC, HPG * (DH + 1)], BF16, tag="v", name=f"v{sc}")
            vv = vt[:].rearrange("p (h e) -> p h e", h=HPG)
            nc.vector.tensor_copy(vv[:, :, DH:DH + 1].squeeze(2), onesb_t[:])
            xsl = slice(sc * 128, (sc + 1) * 128)
            ps = ps_pr.tile([128, 512], F32, tag="proj")
            for dc in range(DC):
                nc.tensor.matmul(ps[:, 0:M], x_t[dc][:, xsl],
                                 w_t[dc][:, 2 * M:3 * M],
                                 start=(dc == 0), stop=(dc == DC - 1))
            nc.vector.tensor_copy(
                vv[:, :, 0:DH],
                ps[:, 0:M].rearrange("p (h d) -> p h d", h=HPG))
            v_t[sc] = vt

        def v_half(sc, half):
            if half == 0:
                vt = p_v.tile([JC, HPG * (DH + 1)], BF16, tag="v",
                              name=f"v{sc}")
                v_t[sc] = vt
            else:
                vt = v_t[sc]
            vv = vt[:].rearrange("p (h e) -> p h e", h=HPG)
            xsl = slice(sc * 128, (sc + 1) * 128)
            dcs = range(DC // 2) if half == 0 else range(DC // 2, DC)
            ps = ps_pr.tile([128, 512], F32, tag="proj")
            for u, dc in enumerate(dcs):
                nc.tensor.matmul(ps[:, 0:M], x_t[dc][:, xsl],
                                 w_t[dc][:, 2 * M:3 * M],
                                 start=(u == 0), stop=(u == DC // 2 - 1))
            if half == 0:
                nc.vector.tensor_copy(vv[:, :, DH:DH + 1].squeeze(2),
                                      onesb_t[:])
                nc.vector.tensor_copy(
                    vv[:, :, 0:DH],
                    ps[:, 0:M].rearrange("p (h d) -> p h d", h=HPG))
            else:
                nc.vector.tensor_add(
                    vv[:, :, 0:DH], vv[:, :, 0:DH],
                    ps[:, 0:M].rearrange("p (h d) -> p h d", h=HPG))

        def oproj_block(sc, tail=False):
            # out[s, o] = sum_k mergedT[k, s] woT[k, o] for s-chunk sc.
            # Mid-kernel: ps_pr slots, DVE evictions. Tail: the scores pool
            # is idle, so use its slots and the idle ACT engine for half the
            # evictions -- keeps the PE streaming through the tail.
            stg = p_ostg.tile([128, D], BF16, tag="ostg")
            for nn in range(2):
                if tail:
                    ps = ps_sc.tile([128, 512], F32, tag="sc", name="ot")
                else:
                    ps = ps_pr.tile([128, 512], F32, tag="proj")
                for kc in range(2):
                    nc.tensor.matmul(
                        ps[:], mg_t[kc][:, sc * 128:(sc + 1) * 128],
                        wo_t[kc][:, nn * 512:(nn + 1) * 512],
                        start=(kc == 0), stop=(kc == 1))
                if nn == 0:
                    nc.vector.tensor_copy(stg[:, 0:512], ps[:])
                else:
                    nc.scalar.copy(stg[:, 512:1024], ps[:])
            nc.sync.dma_start(out_d[sc * 128:(sc + 1) * 128, :], stg[:])

        # ---- attention ----
        probs_store = {}   # (h, ic) -> list of probs tiles
        at_store = {}      # (h, ic) -> at_ps handle

        def chunk_list(ic):
            return list(range(4 * ic + 4))

        def score_chunk(h, ic, jc):
            qk_tile, prow = h // 2, 64 * (h % 2)
            sc_ps = ps_sc.tile([JC, IC], F32, tag="sc", name="scps")
            pr = p_probs.tile([JC, IC], BF16, tag="probs", name="pr")
            z = max(0, (jc - 4 * ic) * 128)
            nc.tensor.matmul(
                sc_ps[:, z:IC],
                k_t[(qk_tile, jc // 4)][prow:prow + DH,
                                        (jc % 4) * JC:(jc % 4 + 1) * JC],
                q_t[(qk_tile, ic)][prow:prow + DH, z:IC],
                start=True, stop=True)
            d = jc - 4 * ic
            if d >= 1:
                # diagonal chunk at offset d: columns < 128*d are entirely
                # causally dead -- zero them on DVE and exp only the live
                # region (the exp stream paces the attention phase).
                z = d * 128
                nc.vector.memset(pr[:, 0:z], 0.0)
                nc.scalar.activation(pr[:, z:IC], sc_ps[:, z:IC], EXP,
                                     scale=SCALE)
                nc.vector.tensor_mul(pr[:, z:IC], pr[:, z:IC],
                                     mask_t[:, d * IC + z:(d + 1) * IC])
            elif d == 0:
                nc.scalar.activation(pr[:], sc_ps[:], EXP, scale=SCALE)
                nc.vector.tensor_mul(pr[:], pr[:], mask_t[:, 0:IC])
            else:
                nc.scalar.activation(pr[:], sc_ps[:], EXP, scale=SCALE)
            probs_store[(h, ic)].append(pr)

        def av_plan(ic):
            # (probs index, dead-col offset z, start, stop). For ic >= 1 the
            # diagonal chunks run d3..d0 with trimmed [z:512] ranges so the
            # full-width d0 lands last and carries the stop flag.
            if ic == 0:
                return [(u, 0, u == 0, u == 3) for u in range(4)]
            plan = [(u, 0, u == 0, False) for u in range(4 * ic)]
            for d in (3, 2, 1):
                plan.append((4 * ic + d, 128 * d, False, False))
            plan.append((4 * ic, 0, False, True))
            return plan

        def av_chunk(h, ic, c, chunks):
            u, z, start, stop = av_plan(ic)[c]
            if c == 0:
                at_store[(h, ic)] = ps_at.tile([DH + 1, IC], F32, tag="attn",
                                               name=f"at{h}_{ic}")
            nc.tensor.matmul(
                at_store[(h, ic)][:, z:IC],
                v_t[u][:, h * (DH + 1):(h + 1) * (DH + 1)],
                probs_store[(h, ic)][u][:, z:IC],
                start=start, stop=stop)

        prep_store = {}

        def norm_prep(h, ic):
            # denominator -> reciprocal -> bf16 -> broadcast across the 64
            # head partitions (DVE + GPSIMD; no PE involvement)
            at = at_store[(h, ic)]
            den = p_small.tile([1, IC], F32, tag="den")
            nc.vector.tensor_copy(den[:], at[DH:DH + 1, :])
            rc32 = p_small.tile([1, IC], F32, tag="rc32")
            nc.vector.reciprocal_approx_fast(rc32[:], den[:])
            rcb = p_small.tile([1, IC], BF16, tag="rcb")
            nc.vector.tensor_copy(rcb[:], rc32[:])
            bc = p_bc.tile([DH, IC], BF16, tag="bc")
            nc.gpsimd.partition_broadcast(bc[:], rcb[0:1, :])
            prep_store[(h, ic)] = bc

        def normalize_pair(ic, pairidx):
            # heads (2*pairidx, 2*pairidx+1): rows 0..63 / 64..127 of
            # mergedT tile `pairidx`, columns ic*512..
            h0 = 2 * pairidx
            isl = slice(ic * IC, (ic + 1) * IC)
            for u in (0, 1):
                if (h0 + u, ic) not in prep_store:
                    norm_prep(h0 + u, ic)
            for u in (0, 1):
                at = at_store.pop((h0 + u, ic))
                bc = prep_store.pop((h0 + u, ic))
                nc.vector.tensor_mul(
                    mg_t[pairidx][u * DH:(u + 1) * DH, isl],
                    at[0:DH, :], bc[:])
            probs_store.pop((h0, ic))
            probs_store.pop((h0 + 1, ic))

        # ---- schedule ----
        # Pre-attention in two phases over the d-contraction: all dc 0-3
        # halves first (available ~halfway through the input DMA), then all
        # dc 4-7 halves, so the PE streams through the whole DMA ramp.
        for s4 in range(4):
            qk_half("q", 0, s4, 0)
            qk_half("k", 0, s4, 0)
        for sc in range(4):
            v_half(sc, 0)
        for s4 in range(4):
            qk_half("q", 0, s4, 1)
            qk_half("k", 0, s4, 1)
        for sc in range(4):
            v_half(sc, 1)

        # Work queue of PE blocks zipped between score chunks: remaining v
        # chunks now, o-proj blocks pushed as their mergedT columns complete.
        from collections import deque
        work = deque()
        for s4 in range(4):
            work.append(("qk1", ("q", s4)))
            work.append(("qk1", ("k", s4)))
        for sc in range(4, S // JC):
            work.append(("v", sc))
        reserve = []

        def pop_work():
            if not work:
                return
            kind, arg = work.popleft()
            if kind == "v":
                v_block(arg)
            elif kind == "qk1":
                qk_block(arg[0], 1, arg[1], split=False)
            else:
                oproj_block(arg)

        steps = [(ic, h) for ic in range(S // IC) for h in range(HPG)]
        prev = None
        for g in steps:
            ic, h = g
            last = g == steps[-1]
            chunks = chunk_list(ic)
            probs_store[(h, ic)] = []
            if prev is not None:
                chunks_p = chunk_list(prev[0])
            for c in range(len(chunks)):
                score_chunk(h, ic, chunks[c])
                if prev is not None and c < len(chunks_p):
                    av_chunk(prev[1], prev[0], c, chunks_p)
                if last and c >= 6:  # self-zip: shrink the serial tail
                    av_chunk(h, ic, c - 6, chunks)
                if ic == 0 or c % 4 == 1 or (ic >= 2 and c % 4 == 3):
                    pop_work()
            if prev is not None and prev[1] % 2 == 1:
                normalize_pair(prev[0], prev[1] // 2)
                if prev[1] == HPG - 1:
                    for idx, sc in enumerate(
                            range(4 * prev[0], 4 * prev[0] + 4)):
                        if prev[0] == 1 or (prev[0] == 2 and idx < 2):
                            reserve.append(("o", sc))
                        else:
                            work.append(("o", sc))
            elif prev is not None:
                norm_prep(prev[1], prev[0])
            prev = g

        # ---- tail flush: remaining AVs, then the reserved o-proj blocks
        # run while the last pair's normalize chain drains on DVE/GPSIMD.
        ic, h = prev
        chunks_p = chunk_list(ic)
        for c in range(len(chunks_p) - 6, len(chunks_p)):
            av_chunk(h, ic, c, chunks_p)
            if c % 2 == 0 and reserve:
                oproj_block(reserve.pop(0)[1], tail=True)
        normalize_pair(ic, h // 2)
        for kind, sc in reserve:
            oproj_block(sc, tail=True)
        for sc in range(4 * ic, 4 * ic + 4):
            oproj_block(sc, tail=True)
        while work:
            pop_work()


_NC_CACHE = None


def _get_nc():
    global _NC_CACHE
    if _NC_CACHE is None:
        _NC_CACHE = _build_nc()
    return _NC_CACHE


def _causal_mask_tile():
    # mask[j, d*512 + i] = 1.0 if i >= j + 128*d else 0.0, for the four
    # diagonal-chunk offsets d in 0..3.
    j = np.arange(JC)[:, None]
    i = np.arange(IC)[None, :]
    return np.concatenate(
        [(i >= j + 128 * d).astype(np.float32) for d in range(4)],
        axis=1).astype(BF)


def _prepare_in_maps(inputs):
    x = np.asarray(inputs["in_features"], dtype=np.float32)
    wqT = np.asarray(inputs["q_proj_weight"], np.float32).T
    wkT = np.asarray(inputs["k_proj_weight"], np.float32).T
    wvT = np.asarray(inputs["v_proj_weight"], np.float32).T
    woT = np.asarray(inputs["o_proj_weight"], np.float32).T
    xT = [np.ascontiguousarray(x[b].T).astype(BF) for b in range(B)]
    mask = _causal_mask_tile()

    in_maps = []
    for c in range(NCORES):
        b, g = divmod(c, HPG)
        ms = slice(g * M, (g + 1) * M)
        in_maps.append({
            "xT": xT[b],
            "wqkvT": np.ascontiguousarray(
                np.concatenate([wqT[:, ms], wkT[:, ms], wvT[:, ms]],
                               axis=1)).astype(BF),
            "woT": np.ascontiguousarray(woT[ms, :]).astype(BF),
            "mask": mask,
            "ones_b": np.ones((JC, HPG), BF),
        })
    return in_maps


def kernel(q_proj_weight, k_proj_weight, v_proj_weight, o_proj_weight, in_features):
    in_dtype = np.asarray(in_features).dtype
    in_maps = _prepare_in_maps({
        "q_proj_weight": q_proj_weight,
        "k_proj_weight": k_proj_weight,
        "v_proj_weight": v_proj_weight,
        "o_proj_weight": o_proj_weight,
        "in_features": in_features,
    })
    nc = _get_nc()
    res = bass_utils.run_bass_kernel_spmd(nc, in_maps, core_ids=list(range(NCORES)))
    out = np.zeros((B, S, D), dtype=np.float32)
    for c in range(NCORES):
        out[c // HPG] += res.results[c]["out"].astype(np.float32)
    return out.astype(in_dtype)


# revision 37
# speedup vs baseline: 1.0090x; 1.0090x over previous
"""Multi-head self-attention (B=2, S=2048, D=1024, H=16, causal) on 8 NeuronCores.

Sharding: core c = 4*b + g handles batch b and heads 4g..4g+3 (batch x
head-group parallel). Per core (all matmul operands bf16, fp32 PSUM accum):
  - q/k projections in transposed layout  qT/kT [dh, s]  (dh on partitions)
  - v projection in natural layout [s, dh] with a fused ones-column per head
    (gives the softmax denominator for free during the AV matmul)
  - causal attention in scoresT [j, i] orientation, single-chunk [128, 512]
    score tiles: PE scores -> ACT exp (scale=1/8, no max subtraction) ->
    DVE causal mask multiply on diagonal chunks -> PE AV accumulate.
    Emission is software-pipelined at CHUNK granularity: the AV stream of
    step g-1 and projection/o-proj blocks are zipped between the score
    matmuls of step g, so the in-order PE never waits for ACT's exp stream
    and the HAM clock gate stays released.
  - normalization of attnT by the per-query denominator via DVE reciprocal
    -> GPSIMD partition_broadcast -> DVE multiply into mergedT (bf16)
  - partial o-projection out_c = merged_c @ Wo[:, cols_c].T, bf16 staging
Host sums the 4 bf16 partial outputs per batch in f32 (the only cross-core
reduction).

bf16 matmuls run at 1 cycle/row on the TRN2 PE (fp32 modes are 2x slower)
and enable the compiler's fast-weight-load path for 128-col stationaries.
"""

import numpy as np
import ml_dtypes

import concourse.bass as bass
from concourse import bacc
import concourse.mybir as mybir
import concourse.tile as tile
from concourse import bass_utils

F32 = mybir.dt.float32
BF16 = mybir.dt.bfloat16
EXP = mybir.ActivationFunctionType.Exp
BF = ml_dtypes.bfloat16

B, S, D = 2, 2048, 1024
H, DH = 16, 64
NCORES = 8
HPG = 4                  # heads per group (per core)
M = HPG * DH             # 256 per-core head dims
DC = D // 128            # 8 contraction chunks for projections
IC = 512                 # i (query) chunk for attention
JC = 128                 # j (key) chunk for attention
SCALE = 1.0 / np.sqrt(DH)


def _build_nc():
    nc = bacc.Bacc("TRN2", target_bir_lowering=False, debug=False)

    xT_d = nc.dram_tensor("xT", [D, S], BF16, kind="ExternalInput").ap()
    wqkv_d = nc.dram_tensor("wqkvT", [D, 3 * M], BF16, kind="ExternalInput").ap()
    woT_d = nc.dram_tensor("woT", [M, D], BF16, kind="ExternalInput").ap()
    mask_d = nc.dram_tensor("mask", [JC, 4 * IC], BF16, kind="ExternalInput").ap()
    onesb_d = nc.dram_tensor("ones_b", [JC, HPG], BF16, kind="ExternalInput").ap()
    out_d = nc.dram_tensor("out", [S, D], BF16, kind="ExternalOutput").ap()

    with tile.TileContext(nc) as tc:
        _body(tc, xT_d, wqkv_d, woT_d, mask_d, onesb_d, out_d)
    nc.compile()
    return nc


def _body(tc, xT_d, wqkv_d, woT_d, mask_d, onesb_d, out_d):
    nc = tc.nc
    from contextlib import ExitStack
    ctx = ExitStack()
    with ctx:
        p_x = ctx.enter_context(tc.tile_pool(name="x", bufs=DC))
        p_w = ctx.enter_context(tc.tile_pool(name="w", bufs=DC))
        p_wo = ctx.enter_context(tc.tile_pool(name="wo", bufs=2))
        p_qk = ctx.enter_context(tc.tile_pool(name="qk", bufs=4))
        p_v = ctx.enter_context(tc.tile_pool(name="v", bufs=S // JC))
        p_mg = ctx.enter_context(tc.tile_pool(name="mg", bufs=2))
        p_probs = ctx.enter_context(tc.tile_pool(name="probs", bufs=36))
        p_small = ctx.enter_context(tc.tile_pool(name="small", bufs=4))
        p_bc = ctx.enter_context(tc.tile_pool(name="bc", bufs=4))
        p_mask = ctx.enter_context(tc.tile_pool(name="mask", bufs=1))
        p_ostg = ctx.enter_context(tc.tile_pool(name="ostg", bufs=2))
        p_ones = ctx.enter_context(tc.tile_pool(name="ones", bufs=1))

        ps_at = ctx.enter_context(tc.tile_pool(name="psa", bufs=3, space="PSUM"))
        ps_sc = ctx.enter_context(tc.tile_pool(name="pss", bufs=3, space="PSUM"))
        ps_pr = ctx.enter_context(tc.tile_pool(name="psp", bufs=2, space="PSUM"))

        # ---- HAM pre-warm: a burst of discarded matmuls while the first
        # x/w tiles stream in keeps the PE activity monitor busy so the
        # clock gate is released when the real projections start.
        NWARM = 32
        wrm = p_ones.tile([128, 512], BF16, tag="warm")
        nc.vector.memset(wrm[:], 1.0)
        wrm_ps = ps_at.tile([128, 512], F32, tag="attn", name="warmps")
        for r in range(NWARM):
            nc.tensor.matmul(wrm_ps[:], wrm[:, 0:128], wrm[:],
                             start=(r == 0), stop=(r == NWARM - 1))
        nc.scalar.copy(wrm[:, 0:1], wrm_ps[:, 0:1])  # keep alive vs DCE
        # preload the Exp activation table while the PE warms up (the first
        # real exp would otherwise pay the ~1.3us table load mid-pipeline)
        nc.scalar.activation(wrm[:, 1:2], wrm_ps[:, 1:2], EXP, scale=SCALE)

        # ---- input loads, in the order the projection matmuls consume them
        w_t, x_t = [], []
        for dc in range(DC):
            wt = p_w.tile([128, 3 * M], BF16, tag="w")
            nc.sync.dma_start(wt[:], wqkv_d[dc * 128:(dc + 1) * 128, :])
            w_t.append(wt)
            xt = p_x.tile([128, S], BF16, tag="x")
            nc.sync.dma_start(xt[:], xT_d[dc * 128:(dc + 1) * 128, :])
            x_t.append(xt)
        wo_t = []
        for kc in range(2):
            t = p_wo.tile([128, D], BF16, tag="wo")
            nc.sync.dma_start(t[:], woT_d[kc * 128:(kc + 1) * 128, :])
            wo_t.append(t)
        mask_t = p_mask.tile([JC, 4 * IC], BF16, tag="mask")
        nc.sync.dma_start(mask_t[:], mask_d[:])
        onesb_t = p_ones.tile([JC, HPG], BF16, tag="onesb")
        nc.sync.dma_start(onesb_t[:], onesb_d[:])

        # ---- projection building blocks ----
        q_t = {mc: p_qk.tile([128, S], BF16, tag="qk", name=f"qT{mc}")
               for mc in range(2)}
        k_t = {mc: p_qk.tile([128, S], BF16, tag="qk", name=f"kT{mc}")
               for mc in range(2)}
        mg_t = [p_mg.tile([128, S], BF16, tag="mgT", name=f"mg{i}")
                for i in range(M // 128)]

        def qk_block(tg, mc, s4, split):
            # qT/kT [m, s] = sum_d WT[d, m] xT[d, s]; m-chunk mc, s-chunk s4.
            woff = 0 if tg == "q" else M
            dst = (q_t if tg == "q" else k_t)[mc]
            sl = slice(s4 * 512, (s4 + 1) * 512)
            wsl = slice(woff + mc * 128, woff + (mc + 1) * 128)
            ps = ps_pr.tile([128, 512], F32, tag="proj")
            for dc in range(DC):
                nc.tensor.matmul(ps[:], w_t[dc][:, wsl], x_t[dc][:, sl],
                                 start=(dc == 0), stop=(dc == DC - 1))
            nc.vector.tensor_copy(dst[:, sl], ps[:])

        def qk_half(tg, mc, s4, half):
            # Half-contraction (dc 0-3 or 4-7) evicted immediately: lets the
            # PE run on the first-arrived x/w tiles during the DMA ramp.
            woff = 0 if tg == "q" else M
            dst = (q_t if tg == "q" else k_t)[mc]
            sl = slice(s4 * 512, (s4 + 1) * 512)
            wsl = slice(woff + mc * 128, woff + (mc + 1) * 128)
            dcs = range(DC // 2) if half == 0 else range(DC // 2, DC)
            ps = ps_pr.tile([128, 512], F32, tag="proj")
            for u, dc in enumerate(dcs):
                nc.tensor.matmul(ps[:], w_t[dc][:, wsl], x_t[dc][:, sl],
                                 start=(u == 0), stop=(u == DC // 2 - 1))
            if half == 0:
                nc.vector.tensor_copy(dst[:, sl], ps[:])
            else:
                nc.vector.tensor_add(dst[:, sl], dst[:, sl], ps[:])

        v_t = {}

        def v_block(sc):
            # v[s, m] tile for j-chunk sc: per head h cols h*65..h*65+63 = v,
            # col h*65+64 = 1.0 (softmax denominator column)
            vt = p_v.tile([JC, HPG * (DH + 1)], BF16, tag="v", name=f"v{sc}")
            vv = vt[:].rearrange("p (h e) -> p h e", h=HPG)
            nc.vector.tensor_copy(vv[:, :, DH:DH + 1].squeeze(2), onesb_t[:])
            xsl = slice(sc * 128, (sc + 1) * 128)
            ps = ps_pr.tile([128, 512], F32, tag="proj")
            for dc in range(DC):
                nc.tensor.matmul(ps[:, 0:M], x_t[dc][:, xsl],
                                 w_t[dc][:, 2 * M:3 * M],
                                 start=(dc == 0), stop=(dc == DC - 1))
            nc.vector.tensor_copy(
                vv[:, :, 0:DH],
                ps[:, 0:M].rearrange("p (h d) -> p h d", h=HPG))
            v_t[sc] = vt

        def v_half(sc, half):
            if half == 0:
                vt = p_v.tile([JC, HPG * (DH + 1)], BF16, tag="v",
                              name=f"v{sc}")
                v_t[sc] = vt
            else:
                vt = v_t[sc]
            vv = vt[:].rearrange("p (h e) -> p h e", h=HPG)
            xsl = slice(sc * 128, (sc + 1) * 128)
            dcs = range(DC // 2) if half == 0 else range(DC // 2, DC)
            ps = ps_pr.tile([128, 512], F32, tag="proj")
            for u, dc in enumerate(dcs):
                nc.tensor.matmul(ps[:, 0:M], x_t[dc][:, xsl],
                                 w_t[dc][:, 2 * M:3 * M],
                                 start=(u == 0), stop=(u == DC // 2 - 1))
            if half == 0:
                nc.vector.tensor_copy(vv[:, :, DH:DH + 1].squeeze(2),
                                      onesb_t[:])
                nc.vector.tensor_copy(
                    vv[:, :, 0:DH],
                    ps[:, 0:M].rearrange("p (h d) -> p h d", h=HPG))
            else:
                nc.vector.tensor_add(
                    vv[:, :, 0:DH], vv[:, :, 0:DH],
                    ps[:, 0:M].rearrange("p (h d) -> p h d", h=HPG))

        def oproj_block(sc, tail=False):
            # out[s, o] = sum_k mergedT[k, s] woT[k, o] for s-chunk sc.
            # Mid-kernel: ps_pr slots, DVE evictions. Tail: the scores pool
            # is idle, so use its slots and the idle ACT engine for half the
            # evictions -- keeps the PE streaming through the tail.
            stg = p_ostg.tile([128, D], BF16, tag="ostg")
            for nn in range(2):
                if tail:
                    ps = ps_sc.tile([128, 512], F32, tag="sc", name="ot")
                else:
                    ps = ps_pr.tile([128, 512], F32, tag="proj")
                for kc in range(2):
                    nc.tensor.matmul(
                        ps[:], mg_t[kc][:, sc * 128:(sc + 1) * 128],
                        wo_t[kc][:, nn * 512:(nn + 1) * 512],
                        start=(kc == 0), stop=(kc == 1))
                if nn == 0:
                    nc.vector.tensor_copy(stg[:, 0:512], ps[:])
                else:
                    nc.scalar.copy(stg[:, 512:1024], ps[:])
            nc.sync.dma_start(out_d[sc * 128:(sc + 1) * 128, :], stg[:])

        # ---- attention ----
        probs_store = {}   # (h, ic) -> list of probs tiles
        at_store = {}      # (h, ic) -> at_ps handle

        def chunk_list(ic):
            return list(range(4 * ic + 4))

        def score_chunk(h, ic, jc):
            qk_tile, prow = h // 2, 64 * (h % 2)
            sc_ps = ps_sc.tile([JC, IC], F32, tag="sc", name="scps")
            pr = p_probs.tile([JC, IC], BF16, tag="probs", name="pr")
            z = max(0, (jc - 4 * ic) * 128)
            nc.tensor.matmul(
                sc_ps[:, z:IC],
                k_t[qk_tile][prow:prow + DH, jc * JC:(jc + 1) * JC],
                q_t[qk_tile][prow:prow + DH, ic * IC + z:(ic + 1) * IC],
                start=True, stop=True)
            d = jc - 4 * ic
            if d >= 1:
                # diagonal chunk at offset d: columns < 128*d are entirely
                # causally dead -- zero them on DVE and exp only the live
                # region (the exp stream paces the attention phase).
                z = d * 128
                nc.vector.memset(pr[:, 0:z], 0.0)
                nc.scalar.activation(pr[:, z:IC], sc_ps[:, z:IC], EXP,
                                     scale=SCALE)
                nc.vector.tensor_mul(pr[:, z:IC], pr[:, z:IC],
                                     mask_t[:, d * IC + z:(d + 1) * IC])
            elif d == 0:
                nc.scalar.activation(pr[:], sc_ps[:], EXP, scale=SCALE)
                nc.vector.tensor_mul(pr[:], pr[:], mask_t[:, 0:IC])
            else:
                nc.scalar.activation(pr[:], sc_ps[:], EXP, scale=SCALE)
            probs_store[(h, ic)].append(pr)

        def av_plan(ic):
            # (probs index, dead-col offset z, start, stop). For ic >= 1 the
            # diagonal chunks run d3..d0 with trimmed [z:512] ranges so the
            # full-width d0 lands last and carries the stop flag.
            if ic == 0:
                return [(u, 0, u == 0, u == 3) for u in range(4)]
            plan = [(u, 0, u == 0, False) for u in range(4 * ic)]
            for d in (3, 2, 1):
                plan.append((4 * ic + d, 128 * d, False, False))
            plan.append((4 * ic, 0, False, True))
            return plan

        def av_chunk(h, ic, c, chunks):
            u, z, start, stop = av_plan(ic)[c]
            if c == 0:
                at_store[(h, ic)] = ps_at.tile([DH + 1, IC], F32, tag="attn",
                                               name=f"at{h}_{ic}")
            nc.tensor.matmul(
                at_store[(h, ic)][:, z:IC],
                v_t[u][:, h * (DH + 1):(h + 1) * (DH + 1)],
                probs_store[(h, ic)][u][:, z:IC],
                start=start, stop=stop)

        prep_store = {}

        def norm_prep(h, ic):
            # denominator -> reciprocal -> bf16 -> broadcast across the 64
            # head partitions (DVE + GPSIMD; no PE involvement)
            at = at_store[(h, ic)]
            den = p_small.tile([1, IC], F32, tag="den")
            nc.vector.tensor_copy(den[:], at[DH:DH + 1, :])
            rc32 = p_small.tile([1, IC], F32, tag="rc32")
            nc.vector.reciprocal_approx_fast(rc32[:], den[:])
            rcb = p_small.tile([1, IC], BF16, tag="rcb")
            nc.vector.tensor_copy(rcb[:], rc32[:])
            bc = p_bc.tile([DH, IC], BF16, tag="bc")
            nc.gpsimd.partition_broadcast(bc[:], rcb[0:1, :])
            prep_store[(h, ic)] = bc

        def normalize_pair(ic, pairidx):
            # heads (2*pairidx, 2*pairidx+1): rows 0..63 / 64..127 of
            # mergedT tile `pairidx`, columns ic*512..
            h0 = 2 * pairidx
            isl = slice(ic * IC, (ic + 1) * IC)
            for u in (0, 1):
                if (h0 + u, ic) not in prep_store:
                    norm_prep(h0 + u, ic)
            for u in (0, 1):
                at = at_store.pop((h0 + u, ic))
                bc = prep_store.pop((h0 + u, ic))
                nc.vector.tensor_mul(
                    mg_t[pairidx][u * DH:(u + 1) * DH, isl],
                    at[0:DH, :], bc[:])
            probs_store.pop((h0, ic))
            probs_store.pop((h0 + 1, ic))

        # ---- schedule ----
        # Pre-attention in two phases over the d-contraction: all dc 0-3
        # halves first (available ~halfway through the input DMA), then all
        # dc 4-7 halves, so the PE streams through the whole DMA ramp.
        for s4 in range(4):
            qk_half("q", 0, s4, 0)
            qk_half("k", 0, s4, 0)
        for sc in range(4):
            v_half(sc, 0)
        for s4 in range(4):
            qk_half("q", 0, s4, 1)
            qk_half("k", 0, s4, 1)
        for sc in range(4):
            v_half(sc, 1)

        # Work queue of PE blocks zipped between score chunks: remaining v
        # chunks now, o-proj blocks pushed as their mergedT columns complete.
        from collections import deque
        work = deque()
        for s4 in range(4):
            work.append(("qk1", ("q", s4)))
            work.append(("qk1", ("k", s4)))
        for sc in range(4, S // JC):
            work.append(("v", sc))
        reserve = []

        def pop_work():
            if not work:
                return
            kind, arg = work.popleft()
            if kind == "v":
                v_block(arg)
            elif kind == "qk1":
                qk_block(arg[0], 1, arg[1], split=False)
            else:
                oproj_block(arg)

        steps = [(ic, h) for ic in range(S // IC) for h in range(HPG)]
        prev = None
        for g in steps:
            ic, h = g
            last = g == steps[-1]
            chunks = chunk_list(ic)
            probs_store[(h, ic)] = []
            if prev is not None:
                chunks_p = chunk_list(prev[0])
            for c in range(len(chunks)):
                score_chunk(h, ic, chunks[c])
                if prev is not None and c < len(chunks_p):
                    av_chunk(prev[1], prev[0], c, chunks_p)
                if last and c >= 6:  # self-zip: shrink the serial tail
                    av_chunk(h, ic, c - 6, chunks)
                if ic == 0 or c % 4 == 1 or (ic >= 2 and c % 4 == 3):
                    pop_work()
            if prev is not None and prev[1] % 2 == 1:
                normalize_pair(prev[0], prev[1] // 2)
                if prev[1] == HPG - 1:
                    dest = reserve if prev[0] == 1 else work
                    for sc in range(4 * prev[0], 4 * prev[0] + 4):
                        dest.append(("o", sc))
            elif prev is not None:
                norm_prep(prev[1], prev[0])
            prev = g

        # ---- tail flush: remaining AVs, then the reserved o-proj blocks
        # run while the last pair's normalize chain drains on DVE/GPSIMD.
        ic, h = prev
        chunks_p = chunk_list(ic)
        for c in range(len(chunks_p) - 6, len(chunks_p)):
            av_chunk(h, ic, c, chunks_p)
            if c % 2 == 0 and reserve:
                oproj_block(reserve.pop(0)[1], tail=True)
        normalize_pair(ic, h // 2)
        for kind, sc in reserve:
            oproj_block(sc, tail=True)
        for sc in range(4 * ic, 4 * ic + 4):
            oproj_block(sc, tail=True)
        while work:
            pop_work()


_NC_CACHE = None


def _get_nc():
    global _NC_CACHE
    if _NC_CACHE is None:
        _NC_CACHE = _build_nc()
    return _NC_CACHE


def _causal_mask_tile():
    # mask[j, d*512 + i] = 1.0 if i >= j + 128*d else 0.0, for the four
    # diagonal-chunk offsets d in 0..3.
    j = np.arange(JC)[:, None]
    i = np.arange(IC)[None, :]
    return np.concatenate(
        [(i >= j + 128 * d).astype(np.float32) for d in range(4)],
        axis=1).astype(BF)


def _prepare_in_maps(inputs):
    x = np.asarray(inputs["in_features"], dtype=np.float32)
    wqT = np.asarray(inputs["q_proj_weight"], np.float32).T
    wkT = np.asarray(inputs["k_proj_weight"], np.float32).T
    wvT = np.asarray(inputs["v_proj_weight"], np.float32).T
    woT = np.asarray(inputs["o_proj_weight"], np.float32).T
    xT = [np.ascontiguousarray(x[b].T).astype(BF) for b in range(B)]
    mask = _causal_mask_tile()

    in_maps = []
    for c in range(NCORES):
        b, g = divmod(c, HPG)
        ms = slice(g * M, (g + 1) * M)
        in_maps.append({
            "xT": xT[b],
            "wqkvT": np.ascontiguousarray(
                np.concatenate([wqT[:, ms], wkT[:, ms], wvT[:, ms]],
                               axis=1)).astype(BF),
            "woT": np.ascontiguousarray(woT[ms, :]).astype(BF),
            "mask": mask,
            "ones_b": np.ones((JC, HPG), BF),
        })
    return in_maps


def kernel(q_proj_weight, k_proj_weight, v_proj_weight, o_proj_weight, in_features):
    in_dtype = np.asarray(in_features).dtype
    in_maps = _prepare_in_maps({
        "q_proj_weight": q_proj_weight,
        "k_proj_weight": k_proj_weight,
        "v_proj_weight": v_proj_weight,
        "o_proj_weight": o_proj_weight,
        "in_features": in_features,
    })
    nc = _get_nc()
    res = bass_utils.run_bass_kernel_spmd(nc, in_maps, core_ids=list(range(NCORES)))
    out = np.zeros((B, S, D), dtype=np.float32)
    for c in range(NCORES):
        out[c // HPG] += res.results[c]["out"].astype(np.float32)
    return out.astype(in_dtype)
